# revision 2
# baseline (speedup 1.0000x reference)
"""Distributed CLIP loss on 8 Trainium2 NeuronCores (Bass/Tile), fp8 edition.

Strategy (data-parallel over image rows):
  - Core i owns image rows [2048*i, 2048*(i+1)). It receives its image shard
    transposed (d-major) in fp8-e4m3 (features pre-scaled by ALPHA=16 on the
    host so the e4m3 grid covers them well) plus the FULL text matrix
    transposed in fp8.  Every core runs the identical program on the
    identical txt matrix (no per-core roll needed: the diagonal is handled
    on the host).
  - On device, each core computes its (2048 x 16384) block of
    E = exp(scale' * img8 @ txt8^T + bias) with DoubleRow fp8 matmuls
    (contraction 256/instruction, 2x PE throughput; fp32 PSUM accumulation
    across 3 chunks of the 768-dim contraction) and reduces it on the fly:
      * row sums  (fused into the exp activation's accum_out)     -> zrow
      * row maxes (VectorE reduce over the bf16 E tile)           -> rowmax
      * per-partition column sums/maxes over the 16 row-tiles
        (bf16 VectorE tensor ops, 2x DVE rate)                    -> colsum/colmax
  - The host finishes: partition/core reductions of colsum/colmax, the exact
    fp64 diagonal (B dot products), log-sum-exp assembly, the two CE means,
    and the argmax==label accuracies.  Accuracies are made EXACT despite fp8
    noise by re-checking every row/column whose exact diagonal logit is
    within MARGIN of the device max logit (fp8 logit deviation measures
    ~0.02 rms / ~0.11 max; MARGIN=0.4 gives ~4x headroom and only a
    handful of candidate rows, each re-evaluated exactly in fp64).

Loss error budget: fp8 logit noise perturbs log Z by ~3e-4 absolute and the
diagonal is exact, so the loss matches the fp32 reference to ~2e-5 relative
(tolerance is 2e-2).
"""

import math

import numpy as np

import bass_rust
import concourse.bass as bass
import concourse.tile as tile
from concourse import mybir
from concourse.bass_utils import run_bass_kernel_spmd
from concourse.vector_clock import ScopedClock

N_CORES = 8
B = 16384
D = 768
BL = B // N_CORES          # 2048 local image rows per core
W = 2048                   # column-group width (4 PSUM banks, double-buffered)
ALPHA = 16.0               # host prescale before fp8-e4m3 quantization
MARGIN = 0.4               # logit-domain candidate margin for exact acc recheck

FP8 = mybir.dt.float8e4
BF16 = mybir.dt.bfloat16
F32 = mybir.dt.float32

# Row-max strategy: "reduce" = plain 1x TENSOR_REDUCE; "halve" = one bf16
# tensor_max folding [0:W/2] with [W/2:W] (2x rate) then a half-width reduce.
ROWMAX_MODE = "halve"
# Subset strides for the max chains.  A max over a subset of rows/columns can
# only UNDERestimate, which only ADDS accuracy-recheck candidates (never
# misses a true argmax==label hit), so this is unconditionally safe; it cuts
# the DVE passes that were the kernel's bottleneck.  Stride 2 costs ~2x more
# host-rechecked candidates (~30 rows instead of ~15 on this data).
COLMAX_RT_STRIDE = 2   # colmax accumulates every other row-tile
ROWMAX_G_STRIDE = 2    # rowmax computed for every other column-group

_MAXW = 1  # this walrus build allows a single sync-wait per CTRL instruction


def _patched_drain_and_barrier(self, tick_clock, wait_clock):
    """Tail drain with its waits split one-per-instruction (walrus limit)."""
    nc = self.nc
    drain_inst = nc.sync.drain()
    wait_clock.add_sem_waits(
        drain_inst.ins, ScopedClock({None: tick_clock.global_clock})
    )
    si = drain_inst.ins.sync_info
    waits = list(si.on_wait or [])
    if len(waits) > _MAXW:
        si.on_wait = waits[:_MAXW]
        rest = waits[_MAXW:]
        for i in range(0, len(rest), _MAXW):
            extra = nc.sync.drain()
            extra.ins.sync_info = bass_rust.SyncInfo(
                on_wait=rest[i : i + _MAXW], on_update=[]
            )
    nc.all_engine_barrier()
    assert self.sems is not None
    popped = nc._tile_sem_poison_stack.pop()
    assert popped is self._sem_poison
    nc.clear_and_free_semaphores(list(self.sems.allocated().values()))
    nc.all_engine_barrier()


tile.TileContext._drain_and_barrier = _patched_drain_and_barrier

_orig_lower_ordered_insts = tile.TileContext._lower_ordered_insts


def _patched_lower_ordered_insts(self, ordered):
    """Split multi-wait instructions: this walrus build allows one sync-wait
    per ISA instruction, so carry the extras on same-engine NOPs in front."""
    nc = self.nc
    for bb_name, insts in ordered.items():
        new_insts = []
        for inst in insts:
            si = inst.sync_info
            if (
                si is not None
                and si.on_wait
                and len(si.on_wait) > _MAXW
                and inst.engine != mybir.EngineType.Unassigned
            ):
                waits = list(si.on_wait)
                si.on_wait = waits[-_MAXW:]
                carry = waits[: -_MAXW]
                for i in range(0, len(carry), _MAXW):
                    nop = mybir.InstNoOp(
                        name=nc.get_next_instruction_name(),
                        engine=inst.engine,
                        ins=[],
                        outs=[],
                        sync_info=bass_rust.SyncInfo(
                            on_wait=carry[i : i + _MAXW], on_update=[]
                        ),
                    )
                    new_insts.append(nop)
            new_insts.append(inst)
        ordered[bb_name] = new_insts
    return _orig_lower_ordered_insts(self, ordered)


tile.TileContext._lower_ordered_insts = _patched_lower_ordered_insts


def _dedup_ldweights(nc) -> int:
    """Remove back-to-back InstLdweights that reload identical weights.

    Safe because the weights tiles (img_sb) are written once per launch and
    never overwritten, so PE array state stays valid across elided reloads.
    LDWs carrying sync waits/updates are kept; any other PE op resets
    tracking.
    """
    removed = 0
    for f in nc.m.functions:
        for bb in f.blocks:
            insts = list(bb.instructions)
            keep = []
            last_key = None
            changed = False
            for ins in insts:
                tn = type(ins).__name__
                if tn == "InstLdweights":
                    si = ins.sync_info
                    clean = si is None or (not si.on_wait and not si.on_update)
                    key = (
                        str(ins.ins[0]),
                        str(ins.is_transpose),
                        str(getattr(ins, "perf_mode", None)),
                        str(getattr(ins, "tile_position", None)),
                    )
                    if clean and key == last_key:
                        removed += 1
                        changed = True
                        continue
                    last_key = key
                elif tn == "InstMatmult":
                    pass  # matmuls leave the loaded weights untouched
                elif getattr(ins, "engine", None) == mybir.EngineType.PE:
                    last_key = None
                keep.append(ins)
            if changed:
                bb.instructions = keep
    return removed


def build_program(scale_act: float, bias_act: float, reps: int = 1,
                  b: int = B, d: int = D, bl: int = BL, w: int = W) -> bass.Bass:
    """Per-core Bass program (identical on all 8 cores).

    scale_act/bias_act are the exp-activation affine: E = exp(scale_act *
    psum + bias_act) where psum = (ALPHA*img8) @ (ALPHA*txt8)^T.
    reps > 1 repeats the computation for slope-based timing.
    """
    nc = bass.Bass("TRN2", target_bir_lowering=False, debug=False)

    n_g = b // w               # column groups
    n_rt = bl // 128           # row tiles
    n_kk = d // 128            # 128-contraction chunks
    n_k2 = d // 256            # DoubleRow 256-contraction chunks
    nb = w // 512              # PSUM 512-col banks per group

    imgT = nc.dram_tensor("imgT", (d, bl), FP8, kind="ExternalInput").ap()
    txtT = nc.dram_tensor("txtT", (d, b), FP8, kind="ExternalInput").ap()

    colsum_d = nc.dram_tensor("colsum", (n_g, 128, w), BF16, kind="ExternalOutput").ap()
    colmax_d = nc.dram_tensor("colmax", (n_g, 128, w), BF16, kind="ExternalOutput").ap()
    zrow_d = nc.dram_tensor("zrow", (128, n_rt), F32, kind="ExternalOutput").ap()
    rowmax_d = nc.dram_tensor("rowmax", (128, n_rt), F32, kind="ExternalOutput").ap()

    EXP = mybir.ActivationFunctionType.Exp
    X = mybir.AxisListType.X
    DR = mybir.MatmulPerfMode.DoubleRow

    with tile.TileContext(nc) as tc:
        with tc.tile_pool(name="imgp", bufs=1) as imgp, \
             tc.tile_pool(name="txtp", bufs=3) as txtp, \
             tc.tile_pool(name="psum", bufs=2, space="PSUM") as psump, \
             tc.tile_pool(name="ep", bufs=3) as ep, \
             tc.tile_pool(name="scrp", bufs=2) as scrp, \
             tc.tile_pool(name="accs", bufs=2) as accp, \
             tc.tile_pool(name="stats", bufs=1) as statp:

            img_sb = imgp.tile([128, n_kk, bl], FP8)
            for kk in range(n_kk):
                nc.sync.dma_start(
                    img_sb[:, kk, :], imgT[kk * 128 : (kk + 1) * 128, :]
                )

            n_gm = (n_g + ROWMAX_G_STRIDE - 1) // ROWMAX_G_STRIDE
            rowsum_slots = statp.tile([128, n_rt * n_g], F32)
            rowmax_slots = statp.tile([128, n_rt * n_gm], F32)
            zrow_sb = statp.tile([128, n_rt], F32)
            rowmax_sb = statp.tile([128, n_rt], F32)

            for rep in range(reps):
                for g in range(n_g):
                    txt_g = txtp.tile([128, n_kk, w], FP8, tag="txt",
                                      name=f"txt_{rep}_{g}")
                    for kk in range(n_kk):
                        nc.sync.dma_start(
                            txt_g[:, kk, :],
                            txtT[kk * 128 : (kk + 1) * 128, g * w : (g + 1) * w],
                        )
                    colsum_acc = accp.tile([128, w], BF16, tag="cs")
                    colmax_acc = accp.tile([128, w], BF16, tag="cm")
                    for rt in range(n_rt):
                        pb = psump.tile([128, w], F32, tag="pb",
                                        name=f"pb_{rep}_{g}_{rt}")
                        for k2 in range(n_k2):
                            lhsT = img_sb[:, 2 * k2 : 2 * k2 + 2,
                                          rt * 128 : (rt + 1) * 128]
                            for bk in range(nb):
                                nc.tensor.matmul(
                                    pb[:, bk * 512 : (bk + 1) * 512],
                                    lhsT,
                                    txt_g[:, 2 * k2 : 2 * k2 + 2,
                                          bk * 512 : (bk + 1) * 512],
                                    start=(k2 == 0),
                                    stop=(k2 == n_k2 - 1),
                                    perf_mode=DR,
                                )
                        e_t = ep.tile([128, w], BF16, tag="e")
                        s = rt * n_g + g
                        nc.scalar.activation(
                            out=e_t[:],
                            in_=pb[:],
                            func=EXP,
                            scale=scale_act,
                            bias=bias_act,
                            accum_out=rowsum_slots[:, s : s + 1],
                        )
                        if rt == 0:
                            nc.vector.tensor_copy(colsum_acc[:], e_t[:])
                            nc.vector.tensor_copy(colmax_acc[:], e_t[:])
                        else:
                            nc.vector.tensor_add(colsum_acc[:], colsum_acc[:], e_t[:])
                            if rt % COLMAX_RT_STRIDE == 0:
                                nc.vector.tensor_max(
                                    colmax_acc[:], colmax_acc[:], e_t[:]
                                )
                        if g % ROWMAX_G_STRIDE == 0:
                            sm = rt * n_gm + g // ROWMAX_G_STRIDE
                            if ROWMAX_MODE == "halve":
                                # fold halves at 2x TT rate, then reduce half
                                # width: ~1.9us vs 2.3us for the plain reduce
                                scr = scrp.tile([128, w // 2], BF16, tag="scr")
                                nc.vector.tensor_max(
                                    scr[:], e_t[:, : w // 2], e_t[:, w // 2 :]
                                )
                                nc.vector.reduce_max(
                                    out=rowmax_slots[:, sm : sm + 1],
                                    in_=scr[:],
                                    axis=X,
                                )
                            else:
                                nc.vector.reduce_max(
                                    out=rowmax_slots[:, sm : sm + 1],
                                    in_=e_t[:],
                                    axis=X,
                                )
                    nc.sync.dma_start(colsum_d[g], colsum_acc[:])
                    nc.sync.dma_start(colmax_d[g], colmax_acc[:])

                for rt in range(n_rt):
                    nc.vector.reduce_sum(
                        out=zrow_sb[:, rt : rt + 1],
                        in_=rowsum_slots[:, rt * n_g : (rt + 1) * n_g],
                        axis=X,
                    )
                    nc.vector.reduce_max(
                        out=rowmax_sb[:, rt : rt + 1],
                        in_=rowmax_slots[:, rt * n_gm : (rt + 1) * n_gm],
                        axis=X,
                    )
                nc.sync.dma_start(zrow_d, zrow_sb[:])
                nc.sync.dma_start(rowmax_d, rowmax_sb[:])

    _dedup_ldweights(nc)
    return nc


_F8NP = mybir.dt.np(FP8)


def quantize_fp8(x: np.ndarray) -> np.ndarray:
    """alpha-prescaled fp8-e4m3 quantization (clipped to TRN's +-240 range)."""
    return np.clip(np.asarray(x, np.float32) * ALPHA, -240.0, 240.0).astype(_F8NP)


def prepare_inputs(image_features, text_features):
    """Host-side prep: fp8 quantize + transpose; identical txt on every core."""
    img8 = quantize_fp8(image_features)
    txt8 = quantize_fp8(text_features)
    imgT_full = np.ascontiguousarray(img8.T)      # (D, B)
    txtT_full = np.ascontiguousarray(txt8.T)      # (D, B)
    in_maps = []
    for i in range(N_CORES):
        imgT_i = np.ascontiguousarray(imgT_full[:, i * BL : (i + 1) * BL])
        in_maps.append({"imgT": imgT_i, "txtT": txtT_full})
    return in_maps


def compute_scale_bias(image_features, text_features, logit_scale):
    """scale = min(exp(logit_scale), 100); bias keeps exp's argument <= ~70.

    Returns (scale, bias, scale_act): the activation computes
    exp(scale_act * psum + bias) with psum = ALPHA^2 * (img8/ALPHA)@(txt8/ALPHA)^T.
    """
    ls = float(np.asarray(logit_scale))
    scale = 100.0 if ls >= math.log(100.0) else float(math.exp(ls))
    img8 = quantize_fp8(image_features).astype(np.float32) / ALPHA
    txt8 = quantize_fp8(text_features).astype(np.float32) / ALPHA
    ni = float(np.sqrt((img8.astype(np.float64) ** 2).sum(axis=1).max()))
    nt = float(np.sqrt((txt8.astype(np.float64) ** 2).sum(axis=1).max()))
    bound = scale * ni * nt
    bias = -max(0.0, bound - 70.0)
    scale_act = scale / (ALPHA * ALPHA)
    return scale, bias, scale_act


def _exact_diag(img64: np.ndarray, txt64: np.ndarray, scale: float) -> np.ndarray:
    return scale * np.einsum("ij,ij->i", img64, txt64)


def postprocess(results, scale, bias, image_features, text_features):
    """Host-side gather/reduce -> (loss, i2t_acc, t2i_acc).

    Loss from device sums (+ exact fp64 diagonal); accuracies via exact fp64
    recheck of all rows/cols whose exact diagonal is within MARGIN of the
    device (fp8-domain) max logit.
    """
    img64 = np.asarray(image_features, np.float64)
    txt64 = np.asarray(text_features, np.float64)
    diag = _exact_diag(img64, txt64, scale)       # exact diagonal logits

    zrow = np.empty(B, dtype=np.float64)
    rowmax = np.empty(B, dtype=np.float64)
    zcol = np.zeros(B, dtype=np.float64)
    colmax = np.full(B, -np.inf, dtype=np.float64)
    for i, r in enumerate(results):
        # (128, 16) -> local row 128*rt + p
        zrow[i * BL : (i + 1) * BL] = r["zrow"].T.reshape(-1).astype(np.float64)
        rowmax[i * BL : (i + 1) * BL] = r["rowmax"].T.reshape(-1).astype(np.float64)
        # (n_g, 128, W) -> global col g*W + w, partial over partitions
        zcol += r["colsum"].astype(np.float64).sum(axis=1).reshape(-1)
        colmax = np.maximum(colmax, r["colmax"].astype(np.float64).max(axis=1).reshape(-1))

    loss_i2t = np.mean(np.log(zrow) - bias - diag)
    loss_t2i = np.mean(np.log(zcol) - bias - diag)
    loss = (loss_i2t + loss_t2i) / 2.0

    # device-domain max logits
    lm_row = np.log(rowmax) - bias
    lm_col = np.log(colmax) - bias

    cand_r = np.nonzero(diag >= lm_row - MARGIN)[0]
    hits_i2t = 0
    if cand_r.size:
        Lr = scale * (img64[cand_r] @ txt64.T)
        hits_i2t = int((np.argmax(Lr, axis=1) == cand_r).sum())

    cand_c = np.nonzero(diag >= lm_col - MARGIN)[0]
    hits_t2i = 0
    if cand_c.size:
        Lc = scale * (txt64[cand_c] @ img64.T)
        hits_t2i = int((np.argmax(Lc, axis=1) == cand_c).sum())

    return (
        np.float32(loss),
        np.float32(hits_i2t / B),
        np.float32(hits_t2i / B),
    )


_program_cache: dict[tuple[float, float], bass.Bass] = {}


def get_program(scale_act: float, bias: float) -> bass.Bass:
    key = (scale_act, bias)
    if key not in _program_cache:
        _program_cache[key] = build_program(scale_act, bias)
    return _program_cache[key]


def kernel(image_features, text_features, logit_scale):
    scale, bias, scale_act = compute_scale_bias(
        image_features, text_features, logit_scale
    )
    nc = get_program(scale_act, bias)
    in_maps = prepare_inputs(image_features, text_features)
    try:
        res = run_bass_kernel_spmd(nc, in_maps, core_ids=list(range(N_CORES)))
    except Exception:
        # transient accelerator hiccups have been observed on this relay;
        # one retry on a fresh attempt usually clears them
        import time as _time

        _time.sleep(2.0)
        res = run_bass_kernel_spmd(nc, in_maps, core_ids=list(range(N_CORES)))
    return postprocess(res.results, scale, bias, image_features, text_features)


# revision 3
# speedup vs baseline: 1.0692x; 1.0692x over previous
"""Distributed CLIP loss on 8 Trainium2 NeuronCores (Bass/Tile), fp8 edition.

Strategy (data-parallel over image rows):
  - Core i owns image rows [2048*i, 2048*(i+1)). It receives its image shard
    transposed (d-major) in fp8-e4m3 (features pre-scaled by ALPHA=16 on the
    host so the e4m3 grid covers them well) plus the FULL text matrix
    transposed in fp8.  Every core runs the identical program on the
    identical txt matrix (no per-core roll needed: the diagonal is handled
    on the host).
  - On device, each core computes its (2048 x 16384) block of
    E = exp(scale' * img8 @ txt8^T + bias) with DoubleRow fp8 matmuls
    (contraction 256/instruction, 2x PE throughput; fp32 PSUM accumulation
    across 3 chunks of the 768-dim contraction) and reduces it on the fly:
      * row sums  (fused into the exp activation's accum_out)     -> zrow
      * row maxes (VectorE reduce over the bf16 E tile)           -> rowmax
      * per-partition column sums/maxes over the 16 row-tiles
        (bf16 VectorE tensor ops, 2x DVE rate)                    -> colsum/colmax
  - The host finishes: partition/core reductions of colsum/colmax, the exact
    fp64 diagonal (B dot products), log-sum-exp assembly, the two CE means,
    and the argmax==label accuracies.  Accuracies are made EXACT despite fp8
    noise by re-checking every row/column whose exact diagonal logit is
    within MARGIN of the device max logit (fp8 logit deviation measures
    ~0.02 rms / ~0.11 max; MARGIN=0.4 gives ~4x headroom and only a
    handful of candidate rows, each re-evaluated exactly in fp64).

Loss error budget: fp8 logit noise perturbs log Z by ~3e-4 absolute and the
diagonal is exact, so the loss matches the fp32 reference to ~2e-5 relative
(tolerance is 2e-2).
"""

import math

import numpy as np

import bass_rust
import concourse.bass as bass
import concourse.tile as tile
from concourse import mybir
from concourse.bass_utils import run_bass_kernel_spmd
from concourse.vector_clock import ScopedClock

N_CORES = 8
B = 16384
D = 768
BL = B // N_CORES          # 2048 local image rows per core
W = 2048                   # column-group width (4 PSUM banks, double-buffered)
ALPHA = 16.0               # host prescale before fp8-e4m3 quantization
MARGIN = 0.4               # logit-domain candidate margin for exact acc recheck

FP8 = mybir.dt.float8e4
BF16 = mybir.dt.bfloat16
F32 = mybir.dt.float32

# Row-max strategy: "reduce" = plain 1x TENSOR_REDUCE; "halve" = one bf16
# tensor_max folding [0:W/2] with [W/2:W] (2x rate) then a half-width reduce.
ROWMAX_MODE = "halve"
# Subset strides for the max chains.  A max over a subset of rows/columns can
# only UNDERestimate, which only ADDS accuracy-recheck candidates (never
# misses a true argmax==label hit), so this is unconditionally safe; it cuts
# the DVE passes that were the kernel's bottleneck.  Stride 2 costs ~2x more
# host-rechecked candidates (~30 rows instead of ~15 on this data).
COLMAX_RT_STRIDE = 4   # colmax accumulates every 4th row-tile
ROWMAX_G_STRIDE = 4    # rowmax computed for every 4th column-group

_MAXW = 1  # this walrus build allows a single sync-wait per CTRL instruction


def _patched_drain_and_barrier(self, tick_clock, wait_clock):
    """Tail drain with its waits split one-per-instruction (walrus limit)."""
    nc = self.nc
    drain_inst = nc.sync.drain()
    wait_clock.add_sem_waits(
        drain_inst.ins, ScopedClock({None: tick_clock.global_clock})
    )
    si = drain_inst.ins.sync_info
    waits = list(si.on_wait or [])
    if len(waits) > _MAXW:
        si.on_wait = waits[:_MAXW]
        rest = waits[_MAXW:]
        for i in range(0, len(rest), _MAXW):
            extra = nc.sync.drain()
            extra.ins.sync_info = bass_rust.SyncInfo(
                on_wait=rest[i : i + _MAXW], on_update=[]
            )
    nc.all_engine_barrier()
    assert self.sems is not None
    popped = nc._tile_sem_poison_stack.pop()
    assert popped is self._sem_poison
    nc.clear_and_free_semaphores(list(self.sems.allocated().values()))
    nc.all_engine_barrier()


tile.TileContext._drain_and_barrier = _patched_drain_and_barrier

_orig_lower_ordered_insts = tile.TileContext._lower_ordered_insts


def _patched_lower_ordered_insts(self, ordered):
    """Split multi-wait instructions: this walrus build allows one sync-wait
    per ISA instruction, so carry the extras on same-engine NOPs in front."""
    nc = self.nc
    for bb_name, insts in ordered.items():
        new_insts = []
        for inst in insts:
            si = inst.sync_info
            if (
                si is not None
                and si.on_wait
                and len(si.on_wait) > _MAXW
                and inst.engine != mybir.EngineType.Unassigned
            ):
                waits = list(si.on_wait)
                si.on_wait = waits[-_MAXW:]
                carry = waits[: -_MAXW]
                for i in range(0, len(carry), _MAXW):
                    nop = mybir.InstNoOp(
                        name=nc.get_next_instruction_name(),
                        engine=inst.engine,
                        ins=[],
                        outs=[],
                        sync_info=bass_rust.SyncInfo(
                            on_wait=carry[i : i + _MAXW], on_update=[]
                        ),
                    )
                    new_insts.append(nop)
            new_insts.append(inst)
        ordered[bb_name] = new_insts
    return _orig_lower_ordered_insts(self, ordered)


tile.TileContext._lower_ordered_insts = _patched_lower_ordered_insts


def _dedup_ldweights(nc) -> int:
    """Remove back-to-back InstLdweights that reload identical weights.

    Safe because the weights tiles (img_sb) are written once per launch and
    never overwritten, so PE array state stays valid across elided reloads.
    LDWs carrying sync waits/updates are kept; any other PE op resets
    tracking.
    """
    removed = 0
    for f in nc.m.functions:
        for bb in f.blocks:
            insts = list(bb.instructions)
            keep = []
            last_key = None
            changed = False
            for ins in insts:
                tn = type(ins).__name__
                if tn == "InstLdweights":
                    si = ins.sync_info
                    clean = si is None or (not si.on_wait and not si.on_update)
                    key = (
                        str(ins.ins[0]),
                        str(ins.is_transpose),
                        str(getattr(ins, "perf_mode", None)),
                        str(getattr(ins, "tile_position", None)),
                    )
                    if clean and key == last_key:
                        removed += 1
                        changed = True
                        continue
                    last_key = key
                elif tn == "InstMatmult":
                    pass  # matmuls leave the loaded weights untouched
                elif getattr(ins, "engine", None) == mybir.EngineType.PE:
                    last_key = None
                keep.append(ins)
            if changed:
                bb.instructions = keep
    return removed


def build_program(scale_act: float, bias_act: float, reps: int = 1,
                  b: int = B, d: int = D, bl: int = BL, w: int = W) -> bass.Bass:
    """Per-core Bass program (identical on all 8 cores).

    scale_act/bias_act are the exp-activation affine: E = exp(scale_act *
    psum + bias_act) where psum = (ALPHA*img8) @ (ALPHA*txt8)^T.
    reps > 1 repeats the computation for slope-based timing.
    """
    nc = bass.Bass("TRN2", target_bir_lowering=False, debug=False)

    n_g = b // w               # column groups
    n_rt = bl // 128           # row tiles
    n_kk = d // 128            # 128-contraction chunks
    n_k2 = d // 256            # DoubleRow 256-contraction chunks
    nb = w // 512              # PSUM 512-col banks per group

    imgT = nc.dram_tensor("imgT", (d, bl), FP8, kind="ExternalInput").ap()
    txtT = nc.dram_tensor("txtT", (d, b), FP8, kind="ExternalInput").ap()

    colsum_d = nc.dram_tensor("colsum", (n_g, 128, w), BF16, kind="ExternalOutput").ap()
    colmax_d = nc.dram_tensor("colmax", (n_g, 128, w), BF16, kind="ExternalOutput").ap()
    zrow_d = nc.dram_tensor("zrow", (128, n_rt), F32, kind="ExternalOutput").ap()
    rowmax_d = nc.dram_tensor("rowmax", (128, n_rt), F32, kind="ExternalOutput").ap()

    EXP = mybir.ActivationFunctionType.Exp
    X = mybir.AxisListType.X
    DR = mybir.MatmulPerfMode.DoubleRow

    with tile.TileContext(nc) as tc:
        with tc.tile_pool(name="imgp", bufs=1) as imgp, \
             tc.tile_pool(name="txtp", bufs=3) as txtp, \
             tc.tile_pool(name="psum", bufs=2, space="PSUM") as psump, \
             tc.tile_pool(name="ep", bufs=3) as ep, \
             tc.tile_pool(name="scrp", bufs=2) as scrp, \
             tc.tile_pool(name="accs", bufs=2) as accp, \
             tc.tile_pool(name="stats", bufs=1) as statp:

            img_sb = imgp.tile([128, n_kk, bl], FP8)
            for kk in range(n_kk):
                nc.sync.dma_start(
                    img_sb[:, kk, :], imgT[kk * 128 : (kk + 1) * 128, :]
                )

            n_gm = (n_g + ROWMAX_G_STRIDE - 1) // ROWMAX_G_STRIDE
            rowsum_slots = statp.tile([128, n_rt * n_g], F32)
            rowmax_slots = statp.tile([128, n_rt * n_gm], F32)
            zrow_sb = statp.tile([128, n_rt], F32)
            rowmax_sb = statp.tile([128, n_rt], F32)

            for rep in range(reps):
                for g in range(n_g):
                    txt_g = txtp.tile([128, n_kk, w], FP8, tag="txt",
                                      name=f"txt_{rep}_{g}")
                    for kk in range(n_kk):
                        nc.sync.dma_start(
                            txt_g[:, kk, :],
                            txtT[kk * 128 : (kk + 1) * 128, g * w : (g + 1) * w],
                        )
                    colsum_acc = accp.tile([128, w], BF16, tag="cs")
                    colmax_acc = accp.tile([128, w], BF16, tag="cm")
                    for rt in range(n_rt):
                        pb = psump.tile([128, w], F32, tag="pb",
                                        name=f"pb_{rep}_{g}_{rt}")
                        for k2 in range(n_k2):
                            lhsT = img_sb[:, 2 * k2 : 2 * k2 + 2,
                                          rt * 128 : (rt + 1) * 128]
                            for bk in range(nb):
                                nc.tensor.matmul(
                                    pb[:, bk * 512 : (bk + 1) * 512],
                                    lhsT,
                                    txt_g[:, 2 * k2 : 2 * k2 + 2,
                                          bk * 512 : (bk + 1) * 512],
                                    start=(k2 == 0),
                                    stop=(k2 == n_k2 - 1),
                                    perf_mode=DR,
                                )
                        e_t = ep.tile([128, w], BF16, tag="e")
                        s = rt * n_g + g
                        nc.scalar.activation(
                            out=e_t[:],
                            in_=pb[:],
                            func=EXP,
                            scale=scale_act,
                            bias=bias_act,
                            accum_out=rowsum_slots[:, s : s + 1],
                        )
                        if rt == 0:
                            nc.vector.tensor_copy(colsum_acc[:], e_t[:])
                            nc.vector.tensor_copy(colmax_acc[:], e_t[:])
                        else:
                            nc.vector.tensor_add(colsum_acc[:], colsum_acc[:], e_t[:])
                            if rt % COLMAX_RT_STRIDE == 0:
                                nc.vector.tensor_max(
                                    colmax_acc[:], colmax_acc[:], e_t[:]
                                )
                        if g % ROWMAX_G_STRIDE == 0:
                            sm = rt * n_gm + g // ROWMAX_G_STRIDE
                            if ROWMAX_MODE == "halve":
                                # fold halves at 2x TT rate, then reduce half
                                # width: ~1.9us vs 2.3us for the plain reduce
                                scr = scrp.tile([128, w // 2], BF16, tag="scr")
                                nc.vector.tensor_max(
                                    scr[:], e_t[:, : w // 2], e_t[:, w // 2 :]
                                )
                                nc.vector.reduce_max(
                                    out=rowmax_slots[:, sm : sm + 1],
                                    in_=scr[:],
                                    axis=X,
                                )
                            else:
                                nc.vector.reduce_max(
                                    out=rowmax_slots[:, sm : sm + 1],
                                    in_=e_t[:],
                                    axis=X,
                                )
                    nc.sync.dma_start(colsum_d[g], colsum_acc[:])
                    nc.sync.dma_start(colmax_d[g], colmax_acc[:])

                for rt in range(n_rt):
                    nc.vector.reduce_sum(
                        out=zrow_sb[:, rt : rt + 1],
                        in_=rowsum_slots[:, rt * n_g : (rt + 1) * n_g],
                        axis=X,
                    )
                    nc.vector.reduce_max(
                        out=rowmax_sb[:, rt : rt + 1],
                        in_=rowmax_slots[:, rt * n_gm : (rt + 1) * n_gm],
                        axis=X,
                    )
                nc.sync.dma_start(zrow_d, zrow_sb[:])
                nc.sync.dma_start(rowmax_d, rowmax_sb[:])

    _dedup_ldweights(nc)
    return nc


_F8NP = mybir.dt.np(FP8)


def quantize_fp8(x: np.ndarray) -> np.ndarray:
    """alpha-prescaled fp8-e4m3 quantization (clipped to TRN's +-240 range)."""
    return np.clip(np.asarray(x, np.float32) * ALPHA, -240.0, 240.0).astype(_F8NP)


def prepare_inputs(image_features, text_features):
    """Host-side prep: fp8 quantize + transpose; identical txt on every core."""
    img8 = quantize_fp8(image_features)
    txt8 = quantize_fp8(text_features)
    imgT_full = np.ascontiguousarray(img8.T)      # (D, B)
    txtT_full = np.ascontiguousarray(txt8.T)      # (D, B)
    in_maps = []
    for i in range(N_CORES):
        imgT_i = np.ascontiguousarray(imgT_full[:, i * BL : (i + 1) * BL])
        in_maps.append({"imgT": imgT_i, "txtT": txtT_full})
    return in_maps


def compute_scale_bias(image_features, text_features, logit_scale):
    """scale = min(exp(logit_scale), 100); bias keeps exp's argument <= ~70.

    Returns (scale, bias, scale_act): the activation computes
    exp(scale_act * psum + bias) with psum = ALPHA^2 * (img8/ALPHA)@(txt8/ALPHA)^T.
    """
    ls = float(np.asarray(logit_scale))
    scale = 100.0 if ls >= math.log(100.0) else float(math.exp(ls))
    img8 = quantize_fp8(image_features).astype(np.float32) / ALPHA
    txt8 = quantize_fp8(text_features).astype(np.float32) / ALPHA
    ni = float(np.sqrt((img8.astype(np.float64) ** 2).sum(axis=1).max()))
    nt = float(np.sqrt((txt8.astype(np.float64) ** 2).sum(axis=1).max()))
    bound = scale * ni * nt
    bias = -max(0.0, bound - 70.0)
    scale_act = scale / (ALPHA * ALPHA)
    return scale, bias, scale_act


def _exact_diag(img64: np.ndarray, txt64: np.ndarray, scale: float) -> np.ndarray:
    return scale * np.einsum("ij,ij->i", img64, txt64)


def postprocess(results, scale, bias, image_features, text_features):
    """Host-side gather/reduce -> (loss, i2t_acc, t2i_acc).

    Loss from device sums (+ exact fp64 diagonal); accuracies via exact fp64
    recheck of all rows/cols whose exact diagonal is within MARGIN of the
    device (fp8-domain) max logit.
    """
    img64 = np.asarray(image_features, np.float64)
    txt64 = np.asarray(text_features, np.float64)
    diag = _exact_diag(img64, txt64, scale)       # exact diagonal logits

    zrow = np.empty(B, dtype=np.float64)
    rowmax = np.empty(B, dtype=np.float64)
    zcol = np.zeros(B, dtype=np.float64)
    colmax = np.full(B, -np.inf, dtype=np.float64)
    for i, r in enumerate(results):
        # (128, 16) -> local row 128*rt + p
        zrow[i * BL : (i + 1) * BL] = r["zrow"].T.reshape(-1).astype(np.float64)
        rowmax[i * BL : (i + 1) * BL] = r["rowmax"].T.reshape(-1).astype(np.float64)
        # (n_g, 128, W) -> global col g*W + w, partial over partitions
        zcol += r["colsum"].astype(np.float64).sum(axis=1).reshape(-1)
        colmax = np.maximum(colmax, r["colmax"].astype(np.float64).max(axis=1).reshape(-1))

    loss_i2t = np.mean(np.log(zrow) - bias - diag)
    loss_t2i = np.mean(np.log(zcol) - bias - diag)
    loss = (loss_i2t + loss_t2i) / 2.0

    # device-domain max logits
    lm_row = np.log(rowmax) - bias
    lm_col = np.log(colmax) - bias

    cand_r = np.nonzero(diag >= lm_row - MARGIN)[0]
    hits_i2t = 0
    if cand_r.size:
        Lr = scale * (img64[cand_r] @ txt64.T)
        hits_i2t = int((np.argmax(Lr, axis=1) == cand_r).sum())

    cand_c = np.nonzero(diag >= lm_col - MARGIN)[0]
    hits_t2i = 0
    if cand_c.size:
        Lc = scale * (txt64[cand_c] @ img64.T)
        hits_t2i = int((np.argmax(Lc, axis=1) == cand_c).sum())

    return (
        np.float32(loss),
        np.float32(hits_i2t / B),
        np.float32(hits_t2i / B),
    )


_program_cache: dict[tuple[float, float], bass.Bass] = {}


def get_program(scale_act: float, bias: float) -> bass.Bass:
    key = (scale_act, bias)
    if key not in _program_cache:
        _program_cache[key] = build_program(scale_act, bias)
    return _program_cache[key]


def kernel(image_features, text_features, logit_scale):
    scale, bias, scale_act = compute_scale_bias(
        image_features, text_features, logit_scale
    )
    nc = get_program(scale_act, bias)
    in_maps = prepare_inputs(image_features, text_features)
    try:
        res = run_bass_kernel_spmd(nc, in_maps, core_ids=list(range(N_CORES)))
    except Exception:
        # transient accelerator hiccups have been observed on this relay;
        # one retry on a fresh attempt usually clears them
        import time as _time

        _time.sleep(2.0)
        res = run_bass_kernel_spmd(nc, in_maps, core_ids=list(range(N_CORES)))
    return postprocess(res.results, scale, bias, image_features, text_features)


# revision 4
# speedup vs baseline: 1.0784x; 1.0086x over previous
"""Distributed CLIP loss on 8 Trainium2 NeuronCores (Bass/Tile), fp8 edition.

Strategy (data-parallel over image rows):
  - Core i owns image rows [2048*i, 2048*(i+1)). It receives its image shard
    transposed (d-major) in fp8-e4m3 (features pre-scaled by ALPHA=16 on the
    host so the e4m3 grid covers them well) plus the FULL text matrix
    transposed in fp8.  Every core runs the identical program on the
    identical txt matrix (no per-core roll needed: the diagonal is handled
    on the host).
  - On device, each core computes its (2048 x 16384) block of
    E = exp(scale' * img8 @ txt8^T + bias) with DoubleRow fp8 matmuls
    (contraction 256/instruction, 2x PE throughput; fp32 PSUM accumulation
    across 3 chunks of the 768-dim contraction) and reduces it on the fly:
      * row sums  (fused into the exp activation's accum_out)     -> zrow
      * row maxes (VectorE reduce over the bf16 E tile)           -> rowmax
      * per-partition column sums/maxes over the 16 row-tiles
        (bf16 VectorE tensor ops, 2x DVE rate)                    -> colsum/colmax
  - The host finishes: partition/core reductions of colsum/colmax, the exact
    fp64 diagonal (B dot products), log-sum-exp assembly, the two CE means,
    and the argmax==label accuracies.  Accuracies are made EXACT despite fp8
    noise by re-checking every row/column whose exact diagonal logit is
    within MARGIN of the device max logit (fp8 logit deviation measures
    ~0.02 rms / ~0.11 max; MARGIN=0.4 gives ~4x headroom and only a
    handful of candidate rows, each re-evaluated exactly in fp64).

Loss error budget: fp8 logit noise perturbs log Z by ~3e-4 absolute and the
diagonal is exact, so the loss matches the fp32 reference to ~2e-5 relative
(tolerance is 2e-2).
"""

import math

import numpy as np

import bass_rust
import concourse.bass as bass
import concourse.tile as tile
from concourse import mybir
from concourse.bass_utils import run_bass_kernel_spmd
from concourse.vector_clock import ScopedClock

N_CORES = 8
B = 16384
D = 768
BL = B // N_CORES          # 2048 local image rows per core
W = 2048                   # column-group width (4 PSUM banks, double-buffered)
ALPHA = 16.0               # host prescale before fp8-e4m3 quantization
MARGIN = 0.4               # logit-domain candidate margin for exact acc recheck

FP8 = mybir.dt.float8e4
BF16 = mybir.dt.bfloat16
F32 = mybir.dt.float32

# Row-max strategy: "reduce" = plain 1x TENSOR_REDUCE; "halve" = one bf16
# tensor_max folding [0:W/2] with [W/2:W] (2x rate) then a half-width reduce.
ROWMAX_MODE = "halve"
# Subset strides for the max chains.  A max over a subset of rows/columns can
# only UNDERestimate, which only ADDS accuracy-recheck candidates (never
# misses a true argmax==label hit), so this is unconditionally safe; it cuts
# the DVE passes that were the kernel's bottleneck.  Stride 2 costs ~2x more
# host-rechecked candidates (~30 rows instead of ~15 on this data).
COLMAX_RT_STRIDE = 4   # colmax accumulates every 4th row-tile
ROWMAX_G_STRIDE = 4    # rowmax computed for every 4th column-group

_MAXW = 1  # this walrus build allows a single sync-wait per CTRL instruction


def _patched_drain_and_barrier(self, tick_clock, wait_clock):
    """Tail drain with its waits split one-per-instruction (walrus limit)."""
    nc = self.nc
    drain_inst = nc.sync.drain()
    wait_clock.add_sem_waits(
        drain_inst.ins, ScopedClock({None: tick_clock.global_clock})
    )
    si = drain_inst.ins.sync_info
    waits = list(si.on_wait or [])
    if len(waits) > _MAXW:
        si.on_wait = waits[:_MAXW]
        rest = waits[_MAXW:]
        for i in range(0, len(rest), _MAXW):
            extra = nc.sync.drain()
            extra.ins.sync_info = bass_rust.SyncInfo(
                on_wait=rest[i : i + _MAXW], on_update=[]
            )
    nc.all_engine_barrier()
    assert self.sems is not None
    popped = nc._tile_sem_poison_stack.pop()
    assert popped is self._sem_poison
    nc.clear_and_free_semaphores(list(self.sems.allocated().values()))
    nc.all_engine_barrier()


tile.TileContext._drain_and_barrier = _patched_drain_and_barrier

_orig_lower_ordered_insts = tile.TileContext._lower_ordered_insts


def _patched_lower_ordered_insts(self, ordered):
    """Split multi-wait instructions: this walrus build allows one sync-wait
    per ISA instruction, so carry the extras on same-engine NOPs in front."""
    nc = self.nc
    for bb_name, insts in ordered.items():
        new_insts = []
        for inst in insts:
            si = inst.sync_info
            if (
                si is not None
                and si.on_wait
                and len(si.on_wait) > _MAXW
                and inst.engine != mybir.EngineType.Unassigned
            ):
                waits = list(si.on_wait)
                si.on_wait = waits[-_MAXW:]
                carry = waits[: -_MAXW]
                for i in range(0, len(carry), _MAXW):
                    nop = mybir.InstNoOp(
                        name=nc.get_next_instruction_name(),
                        engine=inst.engine,
                        ins=[],
                        outs=[],
                        sync_info=bass_rust.SyncInfo(
                            on_wait=carry[i : i + _MAXW], on_update=[]
                        ),
                    )
                    new_insts.append(nop)
            new_insts.append(inst)
        ordered[bb_name] = new_insts
    return _orig_lower_ordered_insts(self, ordered)


tile.TileContext._lower_ordered_insts = _patched_lower_ordered_insts


def _dedup_ldweights(nc) -> int:
    """Remove back-to-back InstLdweights that reload identical weights.

    Safe because the weights tiles (img_sb) are written once per launch and
    never overwritten, so PE array state stays valid across elided reloads.
    LDWs carrying sync waits/updates are kept; any other PE op resets
    tracking.
    """
    removed = 0
    for f in nc.m.functions:
        for bb in f.blocks:
            insts = list(bb.instructions)
            keep = []
            last_key = None
            changed = False
            for ins in insts:
                tn = type(ins).__name__
                if tn == "InstLdweights":
                    si = ins.sync_info
                    clean = si is None or (not si.on_wait and not si.on_update)
                    key = (
                        str(ins.ins[0]),
                        str(ins.is_transpose),
                        str(getattr(ins, "perf_mode", None)),
                        str(getattr(ins, "tile_position", None)),
                    )
                    if clean and key == last_key:
                        removed += 1
                        changed = True
                        continue
                    last_key = key
                elif tn == "InstMatmult":
                    pass  # matmuls leave the loaded weights untouched
                elif getattr(ins, "engine", None) == mybir.EngineType.PE:
                    last_key = None
                keep.append(ins)
            if changed:
                bb.instructions = keep
    return removed


def build_program(scale_act: float, bias_act: float, reps: int = 1,
                  b: int = B, d: int = D, bl: int = BL, w: int = W) -> bass.Bass:
    """Per-core Bass program (identical on all 8 cores).

    scale_act/bias_act are the exp-activation affine: E = exp(scale_act *
    psum + bias_act) where psum = (ALPHA*img8) @ (ALPHA*txt8)^T.
    reps > 1 repeats the computation for slope-based timing.
    """
    nc = bass.Bass("TRN2", target_bir_lowering=False, debug=False)

    n_g = b // w               # column groups
    n_rt = bl // 128           # row tiles
    n_kk = d // 128            # 128-contraction chunks
    n_k2 = d // 256            # DoubleRow 256-contraction chunks
    nb = w // 512              # PSUM 512-col banks per group

    imgT = nc.dram_tensor("imgT", (d, bl), FP8, kind="ExternalInput").ap()
    txtT = nc.dram_tensor("txtT", (d, b), FP8, kind="ExternalInput").ap()

    colsum_d = nc.dram_tensor("colsum", (n_g, 128, w), BF16, kind="ExternalOutput").ap()
    colmax_d = nc.dram_tensor("colmax", (n_g, 128, w), BF16, kind="ExternalOutput").ap()
    zrow_d = nc.dram_tensor("zrow", (128, n_rt), F32, kind="ExternalOutput").ap()
    rowmax_d = nc.dram_tensor("rowmax", (128, n_rt), F32, kind="ExternalOutput").ap()

    EXP = mybir.ActivationFunctionType.Exp
    X = mybir.AxisListType.X
    DR = mybir.MatmulPerfMode.DoubleRow

    with tile.TileContext(nc) as tc:
        with tc.tile_pool(name="imgp", bufs=1) as imgp, \
             tc.tile_pool(name="txtp", bufs=3) as txtp, \
             tc.tile_pool(name="psum", bufs=2, space="PSUM") as psump, \
             tc.tile_pool(name="ep", bufs=3) as ep, \
             tc.tile_pool(name="scrp", bufs=2) as scrp, \
             tc.tile_pool(name="accs", bufs=2) as accp, \
             tc.tile_pool(name="stats", bufs=1) as statp:

            img_sb = imgp.tile([128, n_kk, bl], FP8)
            # PE warm-up: ~8 matmuls on a zeroed tile run during the input DMA
            # wait, flipping the HAM clock gate to 8/8 before the real work.
            warm_sb = imgp.tile([128, 512], BF16)
            nc.gpsimd.memset(warm_sb[:], 0.0)
            warm_pb = psump.tile([128, 512], F32, tag="pb", name="pb_warm")
            for _ in range(8):
                nc.tensor.matmul(
                    warm_pb[:], warm_sb[:, :128], warm_sb[:],
                    start=True, stop=True,
                )

            n_gm = (n_g + ROWMAX_G_STRIDE - 1) // ROWMAX_G_STRIDE
            rowsum_slots = statp.tile([128, n_rt * n_g], F32)
            rowmax_slots = statp.tile([128, n_rt * n_gm], F32)
            zrow_sb = statp.tile([128, n_rt], F32)
            rowmax_sb = statp.tile([128, n_rt], F32)

            for rep in range(reps):
                for g in range(n_g):
                    txt_g = txtp.tile([128, n_kk, w], FP8, tag="txt",
                                      name=f"txt_{rep}_{g}")
                    for kk in range(n_kk):
                        nc.sync.dma_start(
                            txt_g[:, kk, :],
                            txtT[kk * 128 : (kk + 1) * 128, g * w : (g + 1) * w],
                        )
                        if rep == 0 and g == 0:
                            # interleave the one-time img load with the first
                            # txt group (chunk-by-chunk) so the first matmuls
                            # start as soon as both kk=0,1 chunks land instead
                            # of waiting for the whole serial img-then-txt
                            # DMA stream (~17us -> ~5us to first matmul)
                            nc.sync.dma_start(
                                img_sb[:, kk, :],
                                imgT[kk * 128 : (kk + 1) * 128, :],
                            )
                    colsum_acc = accp.tile([128, w], BF16, tag="cs")
                    colmax_acc = accp.tile([128, w], BF16, tag="cm")
                    for rt in range(n_rt):
                        pb = psump.tile([128, w], F32, tag="pb",
                                        name=f"pb_{rep}_{g}_{rt}")
                        for k2 in range(n_k2):
                            lhsT = img_sb[:, 2 * k2 : 2 * k2 + 2,
                                          rt * 128 : (rt + 1) * 128]
                            for bk in range(nb):
                                nc.tensor.matmul(
                                    pb[:, bk * 512 : (bk + 1) * 512],
                                    lhsT,
                                    txt_g[:, 2 * k2 : 2 * k2 + 2,
                                          bk * 512 : (bk + 1) * 512],
                                    start=(k2 == 0),
                                    stop=(k2 == n_k2 - 1),
                                    perf_mode=DR,
                                )
                        e_t = ep.tile([128, w], BF16, tag="e")
                        s = rt * n_g + g
                        nc.scalar.activation(
                            out=e_t[:],
                            in_=pb[:],
                            func=EXP,
                            scale=scale_act,
                            bias=bias_act,
                            accum_out=rowsum_slots[:, s : s + 1],
                        )
                        if rt == 0:
                            nc.vector.tensor_copy(colsum_acc[:], e_t[:])
                            nc.vector.tensor_copy(colmax_acc[:], e_t[:])
                        else:
                            nc.vector.tensor_add(colsum_acc[:], colsum_acc[:], e_t[:])
                            if rt % COLMAX_RT_STRIDE == 0:
                                nc.vector.tensor_max(
                                    colmax_acc[:], colmax_acc[:], e_t[:]
                                )
                        if g % ROWMAX_G_STRIDE == 0:
                            sm = rt * n_gm + g // ROWMAX_G_STRIDE
                            if ROWMAX_MODE == "halve":
                                # fold halves at 2x TT rate, then reduce half
                                # width: ~1.9us vs 2.3us for the plain reduce
                                scr = scrp.tile([128, w // 2], BF16, tag="scr")
                                nc.vector.tensor_max(
                                    scr[:], e_t[:, : w // 2], e_t[:, w // 2 :]
                                )
                                nc.vector.reduce_max(
                                    out=rowmax_slots[:, sm : sm + 1],
                                    in_=scr[:],
                                    axis=X,
                                )
                            else:
                                nc.vector.reduce_max(
                                    out=rowmax_slots[:, sm : sm + 1],
                                    in_=e_t[:],
                                    axis=X,
                                )
                        if g == n_g - 1:
                            # this rt's slots are now complete; the final
                            # reduces issued here (DVE FIFO) overlap with the
                            # remaining row-tiles' matmuls instead of
                            # serializing into the kernel tail
                            nc.vector.reduce_sum(
                                out=zrow_sb[:, rt : rt + 1],
                                in_=rowsum_slots[:, rt * n_g : (rt + 1) * n_g],
                                axis=X,
                            )
                            nc.vector.reduce_max(
                                out=rowmax_sb[:, rt : rt + 1],
                                in_=rowmax_slots[:, rt * n_gm : (rt + 1) * n_gm],
                                axis=X,
                            )
                    nc.sync.dma_start(colsum_d[g], colsum_acc[:])
                    nc.sync.dma_start(colmax_d[g], colmax_acc[:])

                nc.sync.dma_start(zrow_d, zrow_sb[:])
                nc.sync.dma_start(rowmax_d, rowmax_sb[:])

    _dedup_ldweights(nc)
    return nc


_F8NP = mybir.dt.np(FP8)


def quantize_fp8(x: np.ndarray) -> np.ndarray:
    """alpha-prescaled fp8-e4m3 quantization (clipped to TRN's +-240 range)."""
    return np.clip(np.asarray(x, np.float32) * ALPHA, -240.0, 240.0).astype(_F8NP)


def prepare_inputs(image_features, text_features):
    """Host-side prep: fp8 quantize + transpose; identical txt on every core."""
    img8 = quantize_fp8(image_features)
    txt8 = quantize_fp8(text_features)
    imgT_full = np.ascontiguousarray(img8.T)      # (D, B)
    txtT_full = np.ascontiguousarray(txt8.T)      # (D, B)
    in_maps = []
    for i in range(N_CORES):
        imgT_i = np.ascontiguousarray(imgT_full[:, i * BL : (i + 1) * BL])
        in_maps.append({"imgT": imgT_i, "txtT": txtT_full})
    return in_maps


def compute_scale_bias(image_features, text_features, logit_scale):
    """scale = min(exp(logit_scale), 100); bias keeps exp's argument <= ~70.

    Returns (scale, bias, scale_act): the activation computes
    exp(scale_act * psum + bias) with psum = ALPHA^2 * (img8/ALPHA)@(txt8/ALPHA)^T.
    """
    ls = float(np.asarray(logit_scale))
    scale = 100.0 if ls >= math.log(100.0) else float(math.exp(ls))
    img8 = quantize_fp8(image_features).astype(np.float32) / ALPHA
    txt8 = quantize_fp8(text_features).astype(np.float32) / ALPHA
    ni = float(np.sqrt((img8.astype(np.float64) ** 2).sum(axis=1).max()))
    nt = float(np.sqrt((txt8.astype(np.float64) ** 2).sum(axis=1).max()))
    bound = scale * ni * nt
    bias = -max(0.0, bound - 70.0)
    scale_act = scale / (ALPHA * ALPHA)
    return scale, bias, scale_act


def _exact_diag(img64: np.ndarray, txt64: np.ndarray, scale: float) -> np.ndarray:
    return scale * np.einsum("ij,ij->i", img64, txt64)


def postprocess(results, scale, bias, image_features, text_features):
    """Host-side gather/reduce -> (loss, i2t_acc, t2i_acc).

    Loss from device sums (+ exact fp64 diagonal); accuracies via exact fp64
    recheck of all rows/cols whose exact diagonal is within MARGIN of the
    device (fp8-domain) max logit.
    """
    img64 = np.asarray(image_features, np.float64)
    txt64 = np.asarray(text_features, np.float64)
    diag = _exact_diag(img64, txt64, scale)       # exact diagonal logits

    zrow = np.empty(B, dtype=np.float64)
    rowmax = np.empty(B, dtype=np.float64)
    zcol = np.zeros(B, dtype=np.float64)
    colmax = np.full(B, -np.inf, dtype=np.float64)
    for i, r in enumerate(results):
        # (128, 16) -> local row 128*rt + p
        zrow[i * BL : (i + 1) * BL] = r["zrow"].T.reshape(-1).astype(np.float64)
        rowmax[i * BL : (i + 1) * BL] = r["rowmax"].T.reshape(-1).astype(np.float64)
        # (n_g, 128, W) -> global col g*W + w, partial over partitions
        zcol += r["colsum"].astype(np.float64).sum(axis=1).reshape(-1)
        colmax = np.maximum(colmax, r["colmax"].astype(np.float64).max(axis=1).reshape(-1))

    loss_i2t = np.mean(np.log(zrow) - bias - diag)
    loss_t2i = np.mean(np.log(zcol) - bias - diag)
    loss = (loss_i2t + loss_t2i) / 2.0

    # device-domain max logits
    lm_row = np.log(rowmax) - bias
    lm_col = np.log(colmax) - bias

    cand_r = np.nonzero(diag >= lm_row - MARGIN)[0]
    hits_i2t = 0
    if cand_r.size:
        Lr = scale * (img64[cand_r] @ txt64.T)
        hits_i2t = int((np.argmax(Lr, axis=1) == cand_r).sum())

    cand_c = np.nonzero(diag >= lm_col - MARGIN)[0]
    hits_t2i = 0
    if cand_c.size:
        Lc = scale * (txt64[cand_c] @ img64.T)
        hits_t2i = int((np.argmax(Lc, axis=1) == cand_c).sum())

    return (
        np.float32(loss),
        np.float32(hits_i2t / B),
        np.float32(hits_t2i / B),
    )


_program_cache: dict[tuple[float, float], bass.Bass] = {}


def get_program(scale_act: float, bias: float) -> bass.Bass:
    key = (scale_act, bias)
    if key not in _program_cache:
        _program_cache[key] = build_program(scale_act, bias)
    return _program_cache[key]


def kernel(image_features, text_features, logit_scale):
    scale, bias, scale_act = compute_scale_bias(
        image_features, text_features, logit_scale
    )
    nc = get_program(scale_act, bias)
    in_maps = prepare_inputs(image_features, text_features)
    try:
        res = run_bass_kernel_spmd(nc, in_maps, core_ids=list(range(N_CORES)))
    except Exception:
        # transient accelerator hiccups have been observed on this relay;
        # one retry on a fresh attempt usually clears them
        import time as _time

        _time.sleep(2.0)
        res = run_bass_kernel_spmd(nc, in_maps, core_ids=list(range(N_CORES)))
    return postprocess(res.results, scale, bias, image_features, text_features)
